# revision 1
# baseline (speedup 1.0000x reference)
"""Deformable-conv Bass kernel for Trainium2, SPMD over 8 NeuronCores.

Sharding: data-parallel over (batch n, image half). Each core computes a
[O, 64, 128] half-sample. Inside a core:
  1. offset conv (3x3, C=64 -> 18) as 9 PSUM-accumulated matmuls per chunk
  2. PE-transpose offsets to pixel-major [w, h, 18]
  3. bilinear coeffs + int32 gather indices on DVE (exact floor via
     round-and-correct, since the DVE f32->i32 cast rounds to nearest)
  4. per-(tap,row) indirect DMA gathers of 2x2 pixel patches from a
     host-prepped zero-padded patch table in DRAM (the padding implements
     the reference's out-of-bounds mask; one descriptor per w-partition)
  5. corner-weighted sum on DVE (coeff broadcast along channels)
  6. PE-transpose the weighted im2col, 5-chunk PSUM-accumulated GEMM with
     w_def, bias add via ScalarE.
"""
import numpy as np

import concourse.bass as bass
import concourse.mybir as mybir
from concourse import bacc
from concourse.bass_utils import run_bass_kernel_spmd
from concourse.masks import make_identity
from concourse.tile import TileContext

N, C, H, W, O = 4, 64, 128, 128, 64
K, KK, PAD = 3, 9, 4
HP = H + 2 * PAD            # 136 padded rows/cols
NPIX_P = HP * HP            # 18496
NWIN = NPIX_P + 4           # patch-table rows (2x2 pixel patches), small slack
HH = H // 2                 # 64 output rows per core
NCORES = 8
CH = 16                     # h-chunk rows in the tap loop
NCH = HH // CH              # 2 chunks
KC = KK * C                 # 576 contraction size
KCP = 640                   # padded to 5*128
NJ = KCP // 128             # 5 GEMM chunks

F32 = mybir.dt.float32
I16 = mybir.dt.int16
I32 = mybir.dt.int32
ALU = mybir.AluOpType
ACT_COPY = mybir.ActivationFunctionType.Copy
ACT_IDENT = mybir.ActivationFunctionType.Identity

_CACHE: dict = {}
DEBUG_TAPS = False  # add intermediate-dump outputs
SKIP_GATHER = False


def _sq(ap):
    """Drop size-1 free dims (keep partition dim) so DMA AP balancing works."""
    dims = [ap.ap[0]] + [d for d in ap.ap[1:] if d[1] != 1]
    if len(dims) == 1:
        dims.append([1, 1])
    return bass.AP(ap.tensor, ap.offset, dims)


def _build():
    nc = bacc.Bacc("TRN2", target_bir_lowering=False, debug=True)

    xp_d = nc.dram_tensor("xp", [NWIN, 4 * C], F32, kind="ExternalInput")
    xc_d = nc.dram_tensor("xc", [C, (HH + 2) * (W + 2)], F32, kind="ExternalInput")
    woff_d = nc.dram_tensor("woff", [C, KK * 18], F32, kind="ExternalInput")
    wdef_d = nc.dram_tensor("wdef", [128, NJ * O], F32, kind="ExternalInput")
    boff_d = nc.dram_tensor("boff", [18, 1], F32, kind="ExternalInput")
    bdef_d = nc.dram_tensor("bdef", [O, 1], F32, kind="ExternalInput")
    tby_d = nc.dram_tensor("tby", [W, HH * KK], F32, kind="ExternalInput")
    tbx_d = nc.dram_tensor("tbx", [W, HH * KK], F32, kind="ExternalInput")
    out_d = nc.dram_tensor("out", [O, HH * W], F32, kind="ExternalOutput")
    if DEBUG_TAPS:
        dbg_offt = nc.dram_tensor("dbg_offt", [W, HH * 18], F32, kind="ExternalOutput")
        dbg_a00 = nc.dram_tensor("dbg_a00", [W, HH * KK], F32, kind="ExternalOutput")
        dbg_idx = nc.dram_tensor("dbg_idx", [W, KK * HH], I32, kind="ExternalOutput")
        dbg_v0 = nc.dram_tensor("dbg_v0", [128, CH * 2 * C], F32, kind="ExternalOutput")
        dbg_s = nc.dram_tensor("dbg_s", [W, CH * KCP], F32, kind="ExternalOutput")


    with TileContext(nc) as tc:
        with (
            tc.tile_pool(name="const", bufs=1) as cpool,
            tc.tile_pool(name="work", bufs=2) as wpool,
            tc.tile_pool(name="coef", bufs=1) as kpool,
            tc.tile_pool(name="psA", bufs=1, space="PSUM") as psA,
            tc.tile_pool(name="psB", bufs=1, space="PSUM") as psB,
            tc.tile_pool(name="vpool", bufs=2) as vpool,
            tc.tile_pool(name="spool", bufs=2) as spool,
            tc.tile_pool(name="psC", bufs=2, space="PSUM") as psC,
            tc.tile_pool(name="psD", bufs=2, space="PSUM") as psD,
        ):
            WOFF = cpool.tile([C, KK, 18], F32)
            nc.sync.dma_start(WOFF[:], woff_d.ap().rearrange("c (k o) -> c k o", k=KK))
            WDEF = cpool.tile([128, NJ, O], F32)
            nc.sync.dma_start(WDEF[:], wdef_d.ap().rearrange("p (j o) -> p j o", j=NJ))
            BOFF = cpool.tile([18, 1], F32)
            nc.sync.dma_start(BOFF[:], boff_d[:])
            BDEF = cpool.tile([O, 1], F32)
            nc.sync.dma_start(BDEF[:], bdef_d[:])
            IDENT = cpool.tile([128, 128], F32)
            make_identity(nc, IDENT[:])
            XC = cpool.tile([C, HH + 2, W + 2], F32)
            nc.sync.dma_start(XC[:], xc_d.ap().rearrange("c (h w) -> c h w", w=W + 2))
            TBY = cpool.tile([W, HH, KK], F32)
            nc.sync.dma_start(TBY[:], tby_d.ap().rearrange("w (h k) -> w h k", k=KK))
            TBX = cpool.tile([W, HH, KK], F32)
            nc.sync.dma_start(TBX[:], tbx_d.ap().rearrange("w (h k) -> w h k", k=KK))

            # per-chunk coefficient + index tiles (distinct tiles per chunk so
            # chunk-0 gathers don't wait on later chunks' coefficient math)
            As = [[cpool.tile([W, CH, KK], F32, name=f"a{j}c{hc}", tag=f"a{j}c{hc}") for j in range(4)]
                  for hc in range(NCH)]
            IDXs = [cpool.tile([W, KK, CH], I32, name=f"idxc{hc}", tag=f"idxc{hc}") for hc in range(NCH)]

            for hc in range(NCH):
                hsl = slice(hc * CH, (hc + 1) * CH)
                # ---- offset conv rows [hc*CH, hc*CH+CH) + transpose ----
                OFFT = kpool.tile([W, CH, 18], F32, tag="offt")
                for sub in range(CH // 4):
                    hh = hc * CH + 4 * sub
                    offp = psA.tile([18, 512], F32)
                    for k in range(KK):
                        ki, kj = k // K, k % K
                        rhs = XC[:, hh + ki:hh + ki + 4, kj:kj + W]
                        nc.tensor.matmul(
                            offp[:], WOFF[:, k, :], rhs,
                            start=(k == 0), stop=(k == KK - 1),
                        )
                    offs = wpool.tile([18, 512], F32, tag="offs")
                    nc.scalar.activation(offs[:], offp[:], ACT_IDENT, bias=BOFF[:])
                    tp = psB.tile([128, 4, 18], F32)
                    for j in range(4):
                        nc.tensor.transpose(
                            out=tp[:, j, :], in_=offs[:, j * W:(j + 1) * W],
                            identity=IDENT[:18, :18],
                        )
                    nc.vector.tensor_copy(OFFT[:, 4 * sub:4 * sub + 4, :], tp[:])

                # ---- bilinear coeffs + gather indices for this chunk ----
                dy = OFFT[:, :, 0::2]
                dx = OFFT[:, :, 1::2]
                sh = [W, CH, KK]
                PY = kpool.tile(sh, F32, tag="py")
                nc.vector.tensor_tensor(PY[:], TBY[:, hsl, :], dy, ALU.add)
                PX = kpool.tile(sh, F32, tag="px")
                nc.vector.tensor_tensor(PX[:], TBX[:, hsl, :], dx, ALU.add)
                # floor(p) = round(p) - (round(p) > p); f32->i32 cast rounds
                RI = kpool.tile(sh, I32, tag="ri")
                RF = kpool.tile(sh, F32, tag="rf")
                G = kpool.tile(sh, F32, tag="g")
                Y0 = kpool.tile(sh, F32, tag="y0")
                WY = kpool.tile(sh, F32, tag="wy")
                X0 = kpool.tile(sh, F32, tag="x0")
                WX = kpool.tile(sh, F32, tag="wx")
                nc.vector.tensor_copy(RI[:], PY[:])
                nc.vector.tensor_copy(RF[:], RI[:])
                nc.vector.tensor_tensor(G[:], RF[:], PY[:], ALU.is_gt)
                nc.vector.tensor_tensor(Y0[:], RF[:], G[:], ALU.subtract)
                nc.vector.tensor_tensor(WY[:], PY[:], Y0[:], ALU.subtract)
                nc.vector.tensor_copy(RI[:], PX[:])
                nc.vector.tensor_copy(RF[:], RI[:])
                nc.vector.tensor_tensor(G[:], RF[:], PX[:], ALU.is_gt)
                nc.vector.tensor_tensor(X0[:], RF[:], G[:], ALU.subtract)
                nc.vector.tensor_tensor(WX[:], PX[:], X0[:], ALU.subtract)
                CY = kpool.tile(sh, F32, tag="cy")
                nc.vector.tensor_scalar(CY[:], WY[:], -1.0, 1.0, ALU.mult, ALU.add)
                CX = kpool.tile(sh, F32, tag="cx")
                nc.vector.tensor_scalar(CX[:], WX[:], -1.0, 1.0, ALU.mult, ALU.add)
                A00c, A01c, A10c, A11c = As[hc]
                nc.vector.tensor_tensor(A00c[:], CY[:], CX[:], ALU.mult)
                nc.vector.tensor_tensor(A01c[:], CY[:], WX[:], ALU.mult)
                nc.vector.tensor_tensor(A10c[:], WY[:], CX[:], ALU.mult)
                nc.vector.tensor_tensor(A11c[:], WY[:], WX[:], ALU.mult)
                IDXF = kpool.tile(sh, F32, tag="idxf")
                nc.vector.tensor_scalar(IDXF[:], Y0[:], float(HP), None, ALU.mult)
                nc.vector.tensor_tensor(IDXF[:], IDXF[:], X0[:], ALU.add)
                nc.vector.tensor_copy(IDXs[hc][:].transpose([0, 2, 1]), IDXF[:])

                # ---- tap loop for this chunk: gather, weight ----
                S = spool.tile([W, CH, KCP], F32)
                nc.vector.memset(S[:, :, KC:], 0.0)
                for k in range(KK):
                    V = vpool.tile([128, CH, 4 * C], F32, tag="v0")
                    for hl in range(CH):
                        nc.gpsimd.indirect_dma_start(
                            out=V[:, hl, :],
                            out_offset=None,
                            in_=xp_d[:],
                            in_offset=bass.IndirectOffsetOnAxis(
                                ap=IDXs[hc][:, k, hl:hl + 1], axis=0,
                            ),
                        )
                    sk = S[:, :, k * C:(k + 1) * C]

                    def bc(a):
                        return a[:, :, k:k + 1].to_broadcast([W, CH, C])

                    TMP = wpool.tile([W, CH, C], F32, tag="wtmp")
                    nc.vector.tensor_tensor(sk, V[:, :, 0:C], bc(A00c), ALU.mult)
                    nc.vector.tensor_tensor(TMP[:], V[:, :, C:2 * C], bc(A01c), ALU.mult)
                    nc.vector.tensor_tensor(sk, sk, TMP[:], ALU.add)
                    nc.vector.tensor_tensor(TMP[:], V[:, :, 2 * C:3 * C], bc(A10c), ALU.mult)
                    nc.vector.tensor_tensor(sk, sk, TMP[:], ALU.add)
                    nc.vector.tensor_tensor(TMP[:], V[:, :, 3 * C:4 * C], bc(A11c), ALU.mult)
                    nc.vector.tensor_tensor(sk, sk, TMP[:], ALU.add)

                # ---- transpose + GEMM per row ----
                OUTC = wpool.tile([O, CH * W], F32, tag="outc")
                for h in range(CH):
                    stp = psC.tile([128, NJ, 128], F32)
                    for j in range(NJ):
                        nc.tensor.transpose(
                            out=stp[:, j, :],
                            in_=S[:, h, j * 128:(j + 1) * 128],
                            identity=IDENT[:],
                        )
                    scp = wpool.tile([128, NJ, 128], F32, tag="scp")
                    nc.scalar.copy(scp[:], stp[:])
                    outp = psD.tile([O, W], F32)
                    for j in range(NJ):
                        nc.tensor.matmul(
                            outp[:], WDEF[:, j, :], scp[:, j, :],
                            start=(j == 0), stop=(j == NJ - 1),
                        )
                    nc.scalar.activation(
                        OUTC[:, h * W:(h + 1) * W], outp[:],
                        ACT_IDENT, bias=BDEF[:],
                    )
                nc.sync.dma_start(
                    out_d[:, hc * CH * W:(hc + 1) * CH * W], OUTC[:]
                )

    nc.compile()
    return nc


def get_nc():
    if "nc" not in _CACHE:
        _CACHE["nc"] = _build()
    return _CACHE["nc"]


def make_core_inputs(x, w_off, b_off, w_def, b_def):
    """Host-side shard prep: layout/pad transforms only."""
    x = np.ascontiguousarray(x, np.float32)
    # w_off [18, C, 3, 3] -> lhsT per tap: woff[c, k, o18]
    woff = np.ascontiguousarray(
        np.transpose(w_off.reshape(2 * KK, C, K * K), (1, 2, 0)).astype(np.float32)
    ).reshape(C, KK * 18)
    wdef_kc = np.zeros((KCP, O), np.float32)
    wdef_kc[:KC] = w_def.reshape(O, C, KK).transpose(2, 1, 0).reshape(KC, O)
    wdef = np.ascontiguousarray(
        wdef_kc.reshape(NJ, 128, O).transpose(1, 0, 2)
    ).reshape(128, NJ * O)
    boff = b_off.reshape(18, 1).astype(np.float32)
    bdef = b_def.reshape(O, 1).astype(np.float32)

    ki = (np.arange(KK) // K).astype(np.float32)
    kj = (np.arange(KK) % K).astype(np.float32)
    wloc = np.arange(W, dtype=np.float32)
    hloc = np.arange(HH, dtype=np.float32)
    tbx = np.broadcast_to(
        wloc[:, None, None] + kj[None, None, :] - 1 + PAD, (W, HH, KK)
    ).astype(np.float32).reshape(W, HH * KK)

    in_maps = []
    for core in range(NCORES):
        n, half = core // 2, core % 2
        h0 = half * HH
        xpim = np.pad(x[n].transpose(1, 2, 0),
                      ((PAD, PAD + 1), (PAD, PAD + 1), (0, 0)))  # [HP+1, HP+1, C]
        patch = np.concatenate(
            [xpim[:HP, :HP], xpim[:HP, 1:HP + 1],
             xpim[1:HP + 1, :HP], xpim[1:HP + 1, 1:HP + 1]], axis=2)
        xpad = np.zeros((NWIN, 4 * C), np.float32)
        xpad[:NPIX_P] = patch.reshape(NPIX_P, 4 * C)
        pad1 = np.pad(x[n], ((0, 0), (1, 1), (1, 1)))
        xc = np.ascontiguousarray(pad1[:, h0:h0 + HH + 2, :]).reshape(
            C, (HH + 2) * (W + 2)
        )
        tby = np.broadcast_to(
            (h0 + hloc[:, None]) + ki[None, :] - 1 + PAD, (W, HH, KK)
        ).astype(np.float32).reshape(W, HH * KK)
        in_maps.append({
            "xp": xpad, "xc": xc, "woff": woff, "wdef": wdef,
            "boff": boff, "bdef": bdef, "tby": tby, "tbx": tbx,
        })
    return in_maps


def assemble(results):
    full = np.zeros((N, O, H, W), np.float32)
    for core in range(NCORES):
        n, half = core // 2, core % 2
        h0 = half * HH
        full[n, :, h0:h0 + HH, :] = results[core]["out"].reshape(O, HH, W)
    return full


def kernel(x, w_off, b_off, w_def, b_def):
    nc = get_nc()
    in_maps = make_core_inputs(x, w_off, b_off, w_def, b_def)
    res = run_bass_kernel_spmd(nc, in_maps, list(range(NCORES)))
    return assemble(res.results)



# revision 5
# speedup vs baseline: 2.3777x; 2.3777x over previous
"""Deformable-conv Bass kernel for Trainium2, SPMD over 8 NeuronCores.

Sharding: data-parallel over (batch n, image half). Each core computes a
[O, 64, 128] half-sample. Inside a core (per 16-row chunk):
  1. offset conv (3x3, C=64 -> 18) as per-(row,tap) PSUM-accumulated
     matmuls with the image row as lhsT -> pixel-major [w, h, 18] output
     directly (no transpose needed)
  2. bilinear coeffs + gather indices on DVE (exact floor via
     round-and-correct); corner coeffs written duplicated-pairs bf16
  3. int16 indices rearranged into the SWDGE-gather wrapped layout
     ([i%16, i//16], replicated to all 8 gpsimd stripes) via 8+7 small
     SBUF->SBUF DMAs
  4. per-(tap,half-chunk) dma_gather (1024 idx) of bf16 2x2-patch rows
     from a zero-padded DRAM patch table (padding implements the
     reference's out-of-bounds mask)
  5. corner weighting on DVE: coeff bf16 expansion (4x packed copy),
     one packed mult, two pairwise adds
  6. PE-transpose of the weighted im2col (bf16), PSUM->SBUF via ScalarE,
     then 5-chunk GEMM with the im2col as stationary and w_def moving ->
     pixel-major [w, h, O] output; bias via DVE broadcast add.
"""
import numpy as np
import ml_dtypes

import concourse.bass as bass
import concourse.mybir as mybir
from concourse import bacc
from concourse.bass_utils import run_bass_kernel_spmd
from concourse.masks import make_identity
from concourse.tile import TileContext

N, C, H, W, O = 4, 64, 128, 128, 64
K, KK, PAD = 3, 9, 4
HP = H + 2 * PAD            # 136 padded rows/cols
NPIX_P = HP * HP            # 18496
NWIN = NPIX_P + 4           # patch-table rows (2x2 pixel patches), small slack
HH = H // 2                 # 64 output rows per core
NCORES = 8
CH = 16                     # h-chunk rows
NCH = HH // CH              # 4 chunks
KC = KK * C                 # 576 contraction size
KCP = 640                   # padded to 5*128
NJ = KCP // 128             # 5 GEMM chunks

F32 = mybir.dt.float32
BF16 = mybir.dt.bfloat16
I16 = mybir.dt.int16
ALU = mybir.AluOpType

_CACHE: dict = {}


def _build():
    nc = bacc.Bacc("TRN2", target_bir_lowering=False, debug=True)

    xp_d = nc.dram_tensor("xp", [NWIN, 4 * C], BF16, kind="ExternalInput")
    xc_d = nc.dram_tensor("xc", [C, (HH + 2) * (W + 2)], BF16, kind="ExternalInput")
    woff_d = nc.dram_tensor("woff", [C, KK * 18], BF16, kind="ExternalInput")
    wdef_d = nc.dram_tensor("wdef", [128, NJ * O], BF16, kind="ExternalInput")
    bdef_d = nc.dram_tensor("bdef", [128, O], F32, kind="ExternalInput")
    tby_d = nc.dram_tensor("tby", [W, HH * KK], F32, kind="ExternalInput")
    tbx_d = nc.dram_tensor("tbx", [W, HH * KK], F32, kind="ExternalInput")
    out_d = nc.dram_tensor("out", [W, HH * O], F32, kind="ExternalOutput")

    with TileContext(nc) as tc:
        with (
            tc.tile_pool(name="const", bufs=1) as cpool,
            tc.tile_pool(name="work", bufs=2) as wpool,
            tc.tile_pool(name="coef", bufs=1) as kpool,
            tc.tile_pool(name="psA", bufs=1, space="PSUM") as psA,
            tc.tile_pool(name="vpool", bufs=2) as vpool,
            tc.tile_pool(name="spool", bufs=2) as spool,
            tc.tile_pool(name="psC", bufs=2, space="PSUM") as psC,
            tc.tile_pool(name="psD", bufs=2, space="PSUM") as psD,
        ):
            WOFF = cpool.tile([C, KK, 18], BF16)
            nc.sync.dma_start(WOFF[:], woff_d.ap().rearrange("c (k o) -> c k o", k=KK))
            WDEF = cpool.tile([128, NJ, O], BF16)
            nc.sync.dma_start(WDEF[:], wdef_d.ap().rearrange("p (j o) -> p j o", j=NJ))
            BDEF = cpool.tile([128, O], F32)
            nc.sync.dma_start(BDEF[:], bdef_d[:])
            IDENT = cpool.tile([128, 128], F32)
            make_identity(nc, IDENT[:])
            IDENTB = cpool.tile([128, 128], BF16)
            nc.vector.tensor_copy(IDENTB[:], IDENT[:])
            XC = cpool.tile([C, HH + 2, W + 2], BF16)
            nc.sync.dma_start(XC[:], xc_d.ap().rearrange("c (h w) -> c h w", w=W + 2))
            TBY = cpool.tile([W, HH, KK], F32)
            nc.sync.dma_start(TBY[:], tby_d.ap().rearrange("w (h k) -> w h k", k=KK))
            TBX = cpool.tile([W, HH, KK], F32)
            nc.sync.dma_start(TBX[:], tbx_d.ap().rearrange("w (h k) -> w h k", k=KK))

            for hc in range(NCH):
                hsl = slice(hc * CH, (hc + 1) * CH)
                # ---- offset conv, pixel-major [w, h, 18] ----
                psOFF = psA.tile([128, CH, 18], F32)
                for h in range(CH):
                    for k in range(KK):
                        ki, kj = k // K, k % K
                        nc.tensor.matmul(
                            psOFF[:, h, :],
                            XC[:, hc * CH + h + ki, kj:kj + W],
                            WOFF[:, k, :],
                            start=(k == 0), stop=(k == KK - 1),
                        )
                OFFT = kpool.tile([W, CH, 18], F32, tag="offt")
                nc.vector.tensor_copy(OFFT[:], psOFF[:])

                # ---- bilinear coeffs + gather indices (fp32) ----
                dy = OFFT[:, :, 0::2]
                dx = OFFT[:, :, 1::2]
                sh = [W, CH, KK]
                PY = kpool.tile(sh, F32, tag="py")
                nc.vector.tensor_tensor(PY[:], TBY[:, hsl, :], dy, ALU.add)
                PX = kpool.tile(sh, F32, tag="px")
                nc.vector.tensor_tensor(PX[:], TBX[:, hsl, :], dx, ALU.add)
                # floor(p) = round(p) - (round(p) > p); f32->i32 cast rounds
                RI = kpool.tile(sh, mybir.dt.int32, tag="ri")
                RF = kpool.tile(sh, F32, tag="rf")
                G = kpool.tile(sh, F32, tag="g")
                Y0 = kpool.tile(sh, F32, tag="y0")
                WY = kpool.tile(sh, F32, tag="wy")
                X0 = kpool.tile(sh, F32, tag="x0")
                WX = kpool.tile(sh, F32, tag="wx")
                nc.vector.tensor_copy(RI[:], PY[:])
                nc.vector.tensor_copy(RF[:], RI[:])
                nc.vector.tensor_tensor(G[:], RF[:], PY[:], ALU.is_gt)
                nc.vector.tensor_tensor(Y0[:], RF[:], G[:], ALU.subtract)
                nc.vector.tensor_tensor(WY[:], PY[:], Y0[:], ALU.subtract)
                nc.vector.tensor_copy(RI[:], PX[:])
                nc.vector.tensor_copy(RF[:], RI[:])
                nc.vector.tensor_tensor(G[:], RF[:], PX[:], ALU.is_gt)
                nc.vector.tensor_tensor(X0[:], RF[:], G[:], ALU.subtract)
                nc.vector.tensor_tensor(WX[:], PX[:], X0[:], ALU.subtract)
                CY = kpool.tile(sh, F32, tag="cy")
                nc.vector.tensor_scalar(CY[:], WY[:], -1.0, 1.0, ALU.mult, ALU.add)
                CX = kpool.tile(sh, F32, tag="cx")
                nc.vector.tensor_scalar(CX[:], WX[:], -1.0, 1.0, ALU.mult, ALU.add)

                # corner coeffs, bf16, duplicated pairs: [w, h, k, corner, 2]
                A4 = kpool.tile([W, CH, KK, 4, 2], BF16, tag="a4")
                for d in range(2):
                    nc.vector.tensor_tensor(A4[:, :, :, 0, d], CY[:], CX[:], ALU.mult)
                    nc.vector.tensor_tensor(A4[:, :, :, 1, d], CY[:], WX[:], ALU.mult)
                    nc.vector.tensor_tensor(A4[:, :, :, 2, d], WY[:], CX[:], ALU.mult)
                    nc.vector.tensor_tensor(A4[:, :, :, 3, d], WY[:], WX[:], ALU.mult)

                IDXF = kpool.tile(sh, F32, tag="idxf")
                nc.vector.tensor_scalar(IDXF[:], Y0[:], float(HP), None, ALU.mult)
                nc.vector.tensor_tensor(IDXF[:], IDXF[:], X0[:], ALU.add)
                IDXF16 = kpool.tile([W, KK, CH], I16, tag="idxf16")
                nc.vector.tensor_copy(IDXF16[:].transpose([0, 2, 1]), IDXF[:])

                # wrapped-16 idx layout for dma_gather: [i%16, k, i//16]
                # with i = h*128 + w  ->  [w%16, k, h*8 + w//16]
                IDX16 = kpool.tile([128, KK, CH, 8], I16, tag="idx16")
                for a in range(8):
                    nc.sync.dma_start(
                        IDX16[0:16, :, :, a], IDXF16[a * 16:(a + 1) * 16, :, :]
                    )
                for r in range(1, 8):
                    nc.sync.dma_start(
                        IDX16[16 * r:16 * (r + 1), :, :, :], IDX16[0:16, :, :, :]
                    )

                # ---- tap loop: gather, weight ----
                S = spool.tile([W, CH, KCP], BF16)
                nc.vector.memset(S[:, :, KC:], 0.0)
                for k in range(KK):
                    VA = vpool.tile([128, CH, 4 * C], BF16, tag="va")
                    # expand coeffs along c (packed-pair source for 4x mode)
                    src = bass.AP(
                        A4.tensor,
                        A4[:, :, k, 0, 0].offset,
                        [A4[:].ap[0], A4[:].ap[1],
                         [2, 4], [0, C // 2], [1, 2]],
                    )
                    dst = bass.AP(
                        VA.tensor, VA[:].offset,
                        [VA[:].ap[0], VA[:].ap[1],
                         [C, 4], [2, C // 2], [1, 2]],
                    )
                    nc.vector.tensor_copy(dst, src)
                    V = vpool.tile([128, CH, 4 * C], BF16, tag="v0")
                    for g in range(2):
                        nc.gpsimd.dma_gather(
                            out_ap=V[:, 8 * g:8 * (g + 1), :],
                            in_ap=xp_d[:],
                            idxs_ap=IDX16[:, k, 8 * g:8 * (g + 1), :],
                            num_idxs=1024,
                            num_idxs_reg=1024,
                            elem_size=4 * C,
                        )
                    nc.vector.tensor_tensor(V[:], V[:], VA[:], ALU.mult)
                    TMP = wpool.tile([W, CH, 2 * C], BF16, tag="wtmp")
                    nc.vector.tensor_tensor(
                        TMP[:], V[:, :, 0:2 * C], V[:, :, 2 * C:4 * C], ALU.add
                    )
                    nc.vector.tensor_tensor(
                        S[:, :, k * C:(k + 1) * C],
                        TMP[:, :, 0:C], TMP[:, :, C:2 * C], ALU.add,
                    )

                # ---- transpose + GEMM per row, pixel-major out [w, h, O] ----
                psOUT = psD.tile([128, CH, O], F32)
                for h in range(CH):
                    stp = psC.tile([128, NJ, 128], BF16)
                    for j in range(NJ):
                        nc.tensor.transpose(
                            out=stp[:, j, :],
                            in_=S[:, h, j * 128:(j + 1) * 128],
                            identity=IDENTB[:],
                        )
                    scp = wpool.tile([128, NJ, 128], BF16, tag="scp")
                    nc.scalar.copy(scp[:], stp[:])
                    for j in range(NJ):
                        nc.tensor.matmul(
                            psOUT[:, h, :], scp[:, j, :], WDEF[:, j, :],
                            start=(j == 0), stop=(j == NJ - 1),
                        )
                OUTC = wpool.tile([W, CH, O], F32, tag="outc")
                bd = bass.AP(
                    BDEF.tensor, BDEF[:].offset,
                    [BDEF[:].ap[0], [0, CH], [1, O]],
                )
                nc.vector.tensor_tensor(OUTC[:], psOUT[:], bd, ALU.add)
                nc.sync.dma_start(
                    out_d[:, hc * CH * O:(hc + 1) * CH * O], OUTC[:]
                )

    nc.compile()
    return nc


def get_nc():
    if "nc" not in _CACHE:
        _CACHE["nc"] = _build()
    return _CACHE["nc"]


def make_core_inputs(x, w_off, b_off, w_def, b_def):
    """Host-side shard prep: layout/pad/cast transforms only."""
    x = np.ascontiguousarray(x, np.float32)
    # w_off [18, C, 3, 3] -> lhsT per tap: woff[c, k, o18]
    woff = np.ascontiguousarray(
        np.transpose(w_off.reshape(2 * KK, C, K * K), (1, 2, 0)).astype(np.float32)
    ).reshape(C, KK * 18).astype(ml_dtypes.bfloat16)
    wdef_kc = np.zeros((KCP, O), np.float32)
    wdef_kc[:KC] = w_def.reshape(O, C, KK).transpose(2, 1, 0).reshape(KC, O)
    wdef = np.ascontiguousarray(
        wdef_kc.reshape(NJ, 128, O).transpose(1, 0, 2)
    ).reshape(128, NJ * O).astype(ml_dtypes.bfloat16)
    bdef = np.broadcast_to(
        b_def.reshape(1, O).astype(np.float32), (128, O)
    ).copy()

    ki = (np.arange(KK) // K).astype(np.float32)
    kj = (np.arange(KK) % K).astype(np.float32)
    wloc = np.arange(W, dtype=np.float32)
    hloc = np.arange(HH, dtype=np.float32)
    boff = b_off.reshape(KK, 2).astype(np.float32)
    tbx = np.broadcast_to(
        wloc[:, None, None] + kj[None, None, :] - 1 + PAD + boff[None, None, :, 1],
        (W, HH, KK),
    ).astype(np.float32).reshape(W, HH * KK)

    in_maps = []
    for core in range(NCORES):
        n, half = core // 2, core % 2
        h0 = half * HH
        xpim = np.pad(x[n].transpose(1, 2, 0),
                      ((PAD, PAD + 1), (PAD, PAD + 1), (0, 0)))  # [HP+1, HP+1, C]
        patch = np.concatenate(
            [xpim[:HP, :HP], xpim[:HP, 1:HP + 1],
             xpim[1:HP + 1, :HP], xpim[1:HP + 1, 1:HP + 1]], axis=2)
        xpad = np.zeros((NWIN, 4 * C), ml_dtypes.bfloat16)
        xpad[:NPIX_P] = patch.reshape(NPIX_P, 4 * C).astype(ml_dtypes.bfloat16)
        pad1 = np.pad(x[n], ((0, 0), (1, 1), (1, 1)))
        xc = np.ascontiguousarray(pad1[:, h0:h0 + HH + 2, :]).reshape(
            C, (HH + 2) * (W + 2)
        ).astype(ml_dtypes.bfloat16)
        tby = np.broadcast_to(
            (h0 + hloc[None, :, None]) + ki[None, None, :] - 1 + PAD
            + boff[None, None, :, 0],
            (W, HH, KK),
        ).astype(np.float32).reshape(W, HH * KK)
        in_maps.append({
            "xp": xpad, "xc": xc, "woff": woff, "wdef": wdef,
            "bdef": bdef, "tby": tby, "tbx": tbx,
        })
    return in_maps


def assemble(results):
    full = np.zeros((N, O, H, W), np.float32)
    for core in range(NCORES):
        n, half = core // 2, core % 2
        h0 = half * HH
        full[n, :, h0:h0 + HH, :] = (
            results[core]["out"].reshape(W, HH, O).transpose(2, 1, 0)
        )
    return full


def kernel(x, w_off, b_off, w_def, b_def):
    nc = get_nc()
    in_maps = make_core_inputs(x, w_off, b_off, w_def, b_def)
    res = run_bass_kernel_spmd(nc, in_maps, list(range(NCORES)))
    return assemble(res.results)
